# revision 87
# baseline (speedup 1.0000x reference)
"""Trainium2 Bass kernel for nn_MultiHeadAttention_45612552683890.

Math: the reference computes
    q = x*W_q; k = x*W_k; v = x*W_v            (broadcast elementwise)
    scores = (q @ k) / sqrt(E)                 # [B,H,I,I]
    attn   = softmax(scores, axis=2)           # normalizes over axis 2 (rows i)
    emb    = sum_i (attn @ v)                  # [B,H,E], sum over axis 2
    out    = emb @ mlp_w.T + mlp_b

Because softmax normalizes over the SAME axis (i) that is summed afterwards,
sum_i attn[b,h,i,j] == 1 for every (b,h,j).  Therefore
    emb[b,h,e] = sum_j v[b,h,j,e] = sum_j x[b,j,e] * W_v[h,j,e]
exactly.  Q/K/softmax are dead computation.  The kernel computes only:
    emb[b,h,e] = sum_j x[b,j,e]*W_v[h,j,e];   out = emb @ mlp_w.T + mlp_b

Sharding: e (embedding axis, 512) is split 8 ways -> 64 columns per core.
Each core computes its rank-64 contribution to the final Linear; the host
sums the 8 partials and adds the bias.

All device data is bf16 (PSUM accumulation fp32): halves DMA traffic vs
fp32 and runs matmuls at 1 cycle/column instead of 4.  End-to-end relative
error vs the fp32 reference is ~3e-3 (gate is 2e-2).

Latency structure (the stream is DMA-transfer-bound at ~360 GB/s; the tail
after the last input byte is pure latency):
  - j-tile-major stream: jt0-2 as one 512KB DMA each, jt3 as four
    e-quarter chunks so stage-1 quarters complete (and drain) as the
    stream ends; quarters are uneven (24/16/16/8 e-columns) so the
    tail-critical final quarter's matmul burst and drain are half-size;
    the aux block (mlp_w^T + identity) ships last since its consumers
    start later than the final j-tile quarter.
  - stage-1 quarter drains split across ACT (q0/q2) and DVE (q1/q3) so
    the tail-critical q3 drain never queues behind another copy.
  - stage 2 is one 16-transpose PE group into a single PSUM bank with a
    single DVE drain (fewer cross-engine ack/semaphore links beats
    nominal parallelism under the per-engine-counter sync scheme).
  - stage 3 is one K=64 pass in N=256 f-chunks (the final chunk split
    2x128 on dedicated banks so its drains are short and parallel); the
    split chunk lands in the last output columns, so the output ships as
    one large store (cols 0:768, whose HWDGE generation overlaps the
    final chunk's compute) plus one 64KB tail store.
"""

import numpy as np

B, H, J, E, F = 16, 16, 512, 512, 512
NCORES = 8
ES = E // NCORES  # 64 e-columns per core
# uneven e-quarters: the LAST quarter is tail-critical (its stage-1 burst
# and drain gate the whole back end), so keep it small
EQS = (24, 16, 16, 8)
EOFF = (0, 24, 40, 56)
JT = J // 128  # 4 j-tiles
_CACHED = {}


def _build_module():
    import concourse.bacc as bacc
    import concourse.mybir as mybir
    from concourse.tile import TileContext

    f32 = mybir.dt.float32
    bf16 = mybir.dt.bfloat16
    nc = bacc.Bacc("TRN2", target_bir_lowering=False, debug=False)

    XWQ = 2 * B * (ES // 4)  # 512 columns per average e-quarter
    AUXW = F + H  # host-transposed mlp_w shard + 16x16 identity
    xwa_d = nc.dram_tensor("xwa", (J, 4 * XWQ + AUXW), bf16, kind="ExternalInput")
    # single output tensor, row r holds out rows r and r+128 concatenated
    # (one store = one HWDGE generation); the host splits the columns
    out_d = nc.dram_tensor("out", (128, 2 * F), bf16, kind="ExternalOutput")

    xwa_ap = xwa_d.ap().rearrange("(jt p) c -> jt p c", p=128)

    with TileContext(nc) as tc:
        with (
            tc.tile_pool(name="load", bufs=1) as lpool,
            tc.tile_pool(name="work", bufs=1) as wpool,
            tc.tile_pool(name="ps_emb", bufs=1, space="PSUM") as ps_emb_pool,
            tc.tile_pool(name="ps_et", bufs=1, space="PSUM") as ps_et_pool,
            tc.tile_pool(name="ps_po", bufs=1, space="PSUM") as ps_po_pool,
        ):
            # ---- loads (HWDGE ring is FIFO: consumption order).  jt0-2
            # as one 512KB DMA each, then jt3 as four 128KB quarters.
            xw = {}
            for jt in range(JT - 1):
                t = lpool.tile([128, 2 * B * ES], bf16, name=f"xw{jt}")
                nc.sync.dma_start(out=t[:], in_=xwa_ap[jt][:, : 4 * XWQ])
                for q in range(4):
                    c0 = 2 * B * EOFF[q]
                    xw[(jt, q)] = t[:, c0 : c0 + 2 * B * EQS[q]].rearrange(
                        "p (s b e) -> p s b e", s=2, e=EQS[q]
                    )

            jt = JT - 1
            for q in range(4):
                t = lpool.tile([128, 2, B, EQS[q]], bf16, name=f"xw{jt}_{q}")
                c0 = 2 * B * EOFF[q]
                nc.sync.dma_start(
                    out=t[:],
                    in_=xwa_ap[jt][:, c0 : c0 + 2 * B * EQS[q]].rearrange(
                        "p (s b e) -> p s b e", s=2, e=EQS[q]
                    ),
                )
                xw[(jt, q)] = t[:]

            # aux (mlp_w^T + identity) loads last: its consumers (transposes
            # and stage 3) start later than the last j-tile quarter anyway
            aux_sb = lpool.tile([ES, AUXW], bf16, name="aux_sb")
            nc.sync.dma_start(out=aux_sb[:], in_=xwa_ap[0][:ES, 4 * XWQ :])
            mlpT = aux_sb[:, :F]  # [e'(64), f(512)]
            ident = aux_sb[:H, F:]

            # ---- stage 1: emb[h, e', b] = sum_j W[h,j,e']*x[b,j,e']
            emb_ps = [
                ps_emb_pool.tile(
                    [H, EQS[q], B], f32, name=f"emb_ps{q}", tag=f"emb_ps{q}"
                )
                for q in range(4)
            ]
            emb_sb = wpool.tile([H, ES, B], bf16)  # [h, e', b]
            for jt in range(JT):
                for q in range(4):
                    for e in range(EQS[q]):
                        src = xw[(jt, q)][:, :, :, e]
                        nc.tensor.matmul(
                            emb_ps[q][:, e, :],
                            lhsT=src[:, 1, :],  # W [128 j, 16 h]
                            rhs=src[:, 0, :],  # x [128 j, 16 b]
                            start=(jt == 0 and e == 0),
                            stop=(jt == JT - 1 and e == EQS[q] - 1),
                            skip_group_check=True,
                        )
                    if jt == JT - 1:
                        # quarter complete: drain fp32 PSUM -> bf16 SBUF
                        dst = emb_sb[:, EOFF[q] : EOFF[q] + EQS[q]]
                        if q in (0, 2):
                            nc.scalar.copy(dst, emb_ps[q][:])
                        else:
                            nc.vector.tensor_copy(out=dst, in_=emb_ps[q][:])

            # ---- stage 2: transpose per b: [h, 64 e'] -> [64 e', h].  One
            # 16-transpose group into a single PSUM bank (no ping-pong
            # stalls), one DVE drain.
            embT = wpool.tile([ES, B, H], bf16)  # rows e', cols b*16+h
            pt = ps_et_pool.tile([ES, B, H], bf16, tag="ps_et")
            for b in range(B):
                nc.tensor.matmul(
                    pt[:, b, :],
                    lhsT=emb_sb[:, :, b],
                    rhs=ident,
                    is_transpose=True,
                    start=(b == 0),
                    stop=(b == B - 1),
                    skip_group_check=True,
                )
            nc.vector.tensor_copy(out=embT[:], in_=pt[:])

            # ---- stage 3: partial_out[bh, f] = embT.T @ mlpT, one K=64
            # pass in N=256 f-chunks (last split 2x128 on its own banks so
            # the final drains are short).  Chunk accumulators reuse the
            # banks freed by the emb quarter drains.
            FH = F // 2
            FQ = F // 4
            ob = wpool.tile([128, 2, 2, FH], bf16, name="ob")  # [bh, mh, fh, f]
            # the split chunk computes last and lands in the last output
            # columns, keeping the tail store small
            for mh in (0, 1):
                for fh in range(2):
                    q = mh * 2 + fh
                    if not (mh == 1 and fh == 1):
                        po = ps_emb_pool.tile(
                            [128, FH], f32, tag=f"emb_ps{q}", name=f"po{q}"
                        )
                        nc.tensor.matmul(
                            po[:],
                            lhsT=embT[:, mh * 8 : (mh + 1) * 8, :],
                            rhs=mlpT[:, fh * FH : (fh + 1) * FH],
                            start=True,
                            stop=True,
                        )
                        if (mh, fh) == (1, 0):
                            nc.scalar.copy(ob[:, mh, fh, :], po[:])
                        else:
                            nc.vector.tensor_copy(out=ob[:, mh, fh, :], in_=po[:])
                    else:
                        # final chunk: two N=128 matmuls on dedicated
                        # banks, drained in parallel on both engines
                        for fq in range(2):
                            po = ps_po_pool.tile([128, FQ], f32, name=f"po3_{fq}")
                            nc.tensor.matmul(
                                po[:],
                                lhsT=embT[:, mh * 8 : (mh + 1) * 8, :],
                                rhs=mlpT[
                                    :, fh * FH + fq * FQ : fh * FH + (fq + 1) * FQ
                                ],
                                start=True,
                                stop=True,
                            )
                            if fq == 0:
                                nc.scalar.copy(
                                    ob[:, mh, fh, fq * FQ : (fq + 1) * FQ], po[:]
                                )
                            else:
                                nc.vector.tensor_copy(
                                    out=ob[:, mh, fh, fq * FQ : (fq + 1) * FQ],
                                    in_=po[:],
                                )
            # two stores: the first covers the three early chunks (cols
            # 0:768) and its HWDGE generation runs while the final split
            # chunk computes; the tail store is only 64KB
            obf = ob[:].rearrange("p m fh f -> p (m fh f)")
            nc.sync.dma_start(out=out_d.ap()[:, : 3 * FH], in_=obf[:, : 3 * FH])
            nc.sync.dma_start(out=out_d.ap()[:, 3 * FH :], in_=obf[:, 3 * FH :])

    nc.compile()
    return nc


def _get_module():
    if "nc" not in _CACHED:
        _CACHED["nc"] = _build_module()
    return _CACHED["nc"]


def _pack_inputs(x, W_v, mlp_w):
    """Host-side shard + pack (bf16) so every DMA source is contiguous."""
    import ml_dtypes

    bf16 = ml_dtypes.bfloat16
    XWQ = 2 * B * (ES // 4)
    AUXW = F + H
    xs = np.asarray(x, dtype=np.float32).reshape(B, J, E).astype(bf16)
    wv = np.asarray(W_v, dtype=np.float32).reshape(H, J, E).astype(bf16)
    mw = np.asarray(mlp_w, dtype=np.float32).astype(bf16)
    ident = np.eye(H, dtype=bf16)
    in_maps = []
    for c in range(NCORES):
        xwa = np.zeros((J, 4 * XWQ + AUXW), dtype=bf16)
        for q in range(4):
            c0 = 2 * B * EOFF[q]
            eq = EQS[q]
            blk = xwa[:, c0 : c0 + 2 * B * eq].reshape(J, 2, B, eq)
            esl = slice(ES * c + EOFF[q], ES * c + EOFF[q] + eq)
            blk[:, 0] = xs[:, :, esl].transpose(1, 0, 2)  # [j, b, e]
            blk[:, 1] = wv[:, :, esl].transpose(1, 0, 2)  # [j, h, e]
        esl = slice(ES * c, ES * (c + 1))
        # mlp_w^T shard straight from the host: rows 0..63 = e', cols = f;
        # the 16x16 transpose identity rides in the trailing columns
        xwa[:ES, 4 * XWQ : 4 * XWQ + F] = mw[:, esl].T
        xwa[:H, 4 * XWQ + F :] = ident
        in_maps.append({"xwa": xwa})
    return in_maps


def run(x, W_v, mlp_w, mlp_b, trace=False, **spmd_kwargs):
    from concourse.bass_utils import run_bass_kernel_spmd

    nc = _get_module()
    in_maps = _pack_inputs(x, W_v, mlp_w)
    res = run_bass_kernel_spmd(
        nc, in_maps, core_ids=list(range(NCORES)), trace=trace, **spmd_kwargs
    )
    partial = np.zeros((B * H, F), dtype=np.float32)
    for r in res.results:
        o = np.asarray(r["out"], dtype=np.float32)
        partial[:128] += o[:, :F]
        partial[128:] += o[:, F:]
    out = partial + np.asarray(mlp_b, dtype=np.float32)[None, :]
    return out.reshape(B, H, F), res


def kernel(x, W_q=None, W_k=None, W_v=None, mlp_w=None, mlp_b=None, **_unused):
    # W_q / W_k are mathematically dead (softmax over the summed axis).
    out, _ = run(x, W_v, mlp_w, mlp_b, trace=False)
    return out


# revision 90
# speedup vs baseline: 1.0066x; 1.0066x over previous
"""Trainium2 Bass kernel for nn_MultiHeadAttention_45612552683890.

Math: the reference computes
    q = x*W_q; k = x*W_k; v = x*W_v            (broadcast elementwise)
    scores = (q @ k) / sqrt(E)                 # [B,H,I,I]
    attn   = softmax(scores, axis=2)           # normalizes over axis 2 (rows i)
    emb    = sum_i (attn @ v)                  # [B,H,E], sum over axis 2
    out    = emb @ mlp_w.T + mlp_b

Because softmax normalizes over the SAME axis (i) that is summed afterwards,
sum_i attn[b,h,i,j] == 1 for every (b,h,j).  Therefore
    emb[b,h,e] = sum_j v[b,h,j,e] = sum_j x[b,j,e] * W_v[h,j,e]
exactly.  Q/K/softmax are dead computation.  The kernel computes only:
    emb[b,h,e] = sum_j x[b,j,e]*W_v[h,j,e];   out = emb @ mlp_w.T + mlp_b

Sharding: e (embedding axis, 512) is split 8 ways -> 64 columns per core.
Each core computes its rank-64 contribution to the final Linear; the host
sums the 8 partials and adds the bias.

All device data is bf16 (PSUM accumulation fp32): halves DMA traffic vs
fp32 and runs matmuls at 1 cycle/column instead of 4.  End-to-end relative
error vs the fp32 reference is ~3e-3 (gate is 2e-2).

Latency structure (the stream is DMA-transfer-bound at ~360 GB/s; the tail
after the last input byte is pure latency):
  - j-tile-major stream: jt0-2 as one 512KB DMA each, jt3 as four
    e-quarter chunks so stage-1 quarters complete (and drain) as the
    stream ends; quarters are uneven (24/16/16/8 e-columns) so the
    tail-critical final quarter's matmul burst and drain are half-size;
    the aux block (mlp_w^T + identity) ships last since its consumers
    start later than the final j-tile quarter.
  - stage-1 quarter drains split across ACT (q0/q2) and DVE (q1/q3) so
    the tail-critical q3 drain never queues behind another copy.
  - stage 2 is one 16-transpose PE group into a single PSUM bank with a
    single DVE drain (fewer cross-engine ack/semaphore links beats
    nominal parallelism under the per-engine-counter sync scheme).
  - stage 3 is one K=64 pass in N=256 f-chunks (the final chunk split
    2x128 on dedicated banks so its drains are short and parallel); the
    split chunk lands in the last output columns, so the output ships as
    one large store (cols 0:768, whose HWDGE generation overlaps the
    final chunk's compute) plus one 64KB tail store.
"""

import numpy as np

B, H, J, E, F = 16, 16, 512, 512, 512
NCORES = 8
ES = E // NCORES  # 64 e-columns per core
# uneven e-quarters: the LAST quarter is tail-critical (its stage-1 burst
# and drain gate the whole back end), so keep it small
EQS = (24, 16, 16, 8)
EOFF = (0, 24, 40, 56)
JT = J // 128  # 4 j-tiles
_CACHED = {}


def _build_module():
    import concourse.bacc as bacc
    import concourse.mybir as mybir
    from concourse.tile import TileContext

    f32 = mybir.dt.float32
    bf16 = mybir.dt.bfloat16
    nc = bacc.Bacc("TRN2", target_bir_lowering=False, debug=False)

    XWQ = 2 * B * (ES // 4)  # 512 columns per average e-quarter
    AUXW = F + H  # host-transposed mlp_w shard + 16x16 identity
    xwa_d = nc.dram_tensor("xwa", (J, 4 * XWQ + AUXW), bf16, kind="ExternalInput")
    # single output tensor, row r holds out rows r and r+128 concatenated
    # (one store = one HWDGE generation); the host splits the columns
    out_d = nc.dram_tensor("out", (128, 2 * F), bf16, kind="ExternalOutput")

    xwa_ap = xwa_d.ap().rearrange("(jt p) c -> jt p c", p=128)

    # jt0 loads BEFORE the TileContext so its HWDGE descriptor generation
    # and transfer overlap the context's semaphore-init/barrier preamble
    # (~650ns).  Manual completion semaphore; the wait is attached to the
    # first PE consumer post-schedule (PE is in-order, so one wait covers
    # every later reader).
    jt0_sb = nc.alloc_sbuf_tensor("jt0_sb", [128, 2 * B * ES], bf16)
    jt0_sem = nc.alloc_semaphore("jt0_dma")
    nc.sync.dma_start(out=jt0_sb.ap(), in_=xwa_ap[0][:, : 4 * XWQ]).then_inc(
        jt0_sem, 16
    )

    with TileContext(nc) as tc:
        with (
            tc.tile_pool(name="load", bufs=1) as lpool,
            tc.tile_pool(name="work", bufs=1) as wpool,
            tc.tile_pool(name="ps_emb", bufs=1, space="PSUM") as ps_emb_pool,
            tc.tile_pool(name="ps_et", bufs=1, space="PSUM") as ps_et_pool,
            tc.tile_pool(name="ps_po", bufs=1, space="PSUM") as ps_po_pool,
        ):
            # ---- loads (HWDGE ring is FIFO: consumption order).  jt0-2
            # as one 512KB DMA each, then jt3 as four 128KB quarters.
            xw = {}
            for q in range(4):
                c0 = 2 * B * EOFF[q]
                xw[(0, q)] = jt0_sb.ap()[:, c0 : c0 + 2 * B * EQS[q]].rearrange(
                    "p (s b e) -> p s b e", s=2, e=EQS[q]
                )
            for jt in range(1, JT - 1):
                t = lpool.tile([128, 2 * B * ES], bf16, name=f"xw{jt}")
                nc.sync.dma_start(out=t[:], in_=xwa_ap[jt][:, : 4 * XWQ])
                for q in range(4):
                    c0 = 2 * B * EOFF[q]
                    xw[(jt, q)] = t[:, c0 : c0 + 2 * B * EQS[q]].rearrange(
                        "p (s b e) -> p s b e", s=2, e=EQS[q]
                    )

            jt = JT - 1
            for q in range(4):
                t = lpool.tile([128, 2, B, EQS[q]], bf16, name=f"xw{jt}_{q}")
                c0 = 2 * B * EOFF[q]
                nc.sync.dma_start(
                    out=t[:],
                    in_=xwa_ap[jt][:, c0 : c0 + 2 * B * EQS[q]].rearrange(
                        "p (s b e) -> p s b e", s=2, e=EQS[q]
                    ),
                )
                xw[(jt, q)] = t[:]

            # aux (mlp_w^T + identity) loads last: its consumers (transposes
            # and stage 3) start later than the last j-tile quarter anyway
            aux_sb = lpool.tile([ES, AUXW], bf16, name="aux_sb")
            nc.sync.dma_start(out=aux_sb[:], in_=xwa_ap[0][:ES, 4 * XWQ :])
            mlpT = aux_sb[:, :F]  # [e'(64), f(512)]
            ident = aux_sb[:H, F:]

            # ---- stage 1: emb[h, e', b] = sum_j W[h,j,e']*x[b,j,e']
            emb_ps = [
                ps_emb_pool.tile(
                    [H, EQS[q], B], f32, name=f"emb_ps{q}", tag=f"emb_ps{q}"
                )
                for q in range(4)
            ]
            emb_sb = wpool.tile([H, ES, B], bf16)  # [h, e', b]
            for jt in range(JT):
                for q in range(4):
                    for e in range(EQS[q]):
                        src = xw[(jt, q)][:, :, :, e]
                        nc.tensor.matmul(
                            emb_ps[q][:, e, :],
                            lhsT=src[:, 1, :],  # W [128 j, 16 h]
                            rhs=src[:, 0, :],  # x [128 j, 16 b]
                            start=(jt == 0 and e == 0),
                            stop=(jt == JT - 1 and e == EQS[q] - 1),
                            skip_group_check=True,
                        )
                    if jt == JT - 1:
                        # quarter complete: drain fp32 PSUM -> bf16 SBUF
                        dst = emb_sb[:, EOFF[q] : EOFF[q] + EQS[q]]
                        if q in (0, 2):
                            nc.scalar.copy(dst, emb_ps[q][:])
                        else:
                            nc.vector.tensor_copy(out=dst, in_=emb_ps[q][:])

            # ---- stage 2: transpose per b: [h, 64 e'] -> [64 e', h].  One
            # 16-transpose group into a single PSUM bank (no ping-pong
            # stalls), one DVE drain.
            embT = wpool.tile([ES, B, H], bf16)  # rows e', cols b*16+h
            pt = ps_et_pool.tile([ES, B, H], bf16, tag="ps_et")
            for b in range(B):
                nc.tensor.matmul(
                    pt[:, b, :],
                    lhsT=emb_sb[:, :, b],
                    rhs=ident,
                    is_transpose=True,
                    start=(b == 0),
                    stop=(b == B - 1),
                    skip_group_check=True,
                )
            nc.vector.tensor_copy(out=embT[:], in_=pt[:])

            # ---- stage 3: partial_out[bh, f] = embT.T @ mlpT, one K=64
            # pass in N=256 f-chunks (last split 2x128 on its own banks so
            # the final drains are short).  Chunk accumulators reuse the
            # banks freed by the emb quarter drains.
            FH = F // 2
            FQ = F // 4
            ob = wpool.tile([128, 2, 2, FH], bf16, name="ob")  # [bh, mh, fh, f]
            # the split chunk computes last and lands in the last output
            # columns, keeping the tail store small
            for mh in (0, 1):
                for fh in range(2):
                    q = mh * 2 + fh
                    if not (mh == 1 and fh == 1):
                        po = ps_emb_pool.tile(
                            [128, FH], f32, tag=f"emb_ps{q}", name=f"po{q}"
                        )
                        nc.tensor.matmul(
                            po[:],
                            lhsT=embT[:, mh * 8 : (mh + 1) * 8, :],
                            rhs=mlpT[:, fh * FH : (fh + 1) * FH],
                            start=True,
                            stop=True,
                        )
                        if (mh, fh) == (1, 0):
                            nc.scalar.copy(ob[:, mh, fh, :], po[:])
                        else:
                            nc.vector.tensor_copy(out=ob[:, mh, fh, :], in_=po[:])
                    else:
                        # final chunk: two N=128 matmuls on dedicated
                        # banks, drained in parallel on both engines
                        for fq in range(2):
                            po = ps_po_pool.tile([128, FQ], f32, name=f"po3_{fq}")
                            nc.tensor.matmul(
                                po[:],
                                lhsT=embT[:, mh * 8 : (mh + 1) * 8, :],
                                rhs=mlpT[
                                    :, fh * FH + fq * FQ : fh * FH + (fq + 1) * FQ
                                ],
                                start=True,
                                stop=True,
                            )
                            if fq == 0:
                                nc.scalar.copy(
                                    ob[:, mh, fh, fq * FQ : (fq + 1) * FQ], po[:]
                                )
                            else:
                                nc.vector.tensor_copy(
                                    out=ob[:, mh, fh, fq * FQ : (fq + 1) * FQ],
                                    in_=po[:],
                                )
            # two stores: the first covers the three early chunks (cols
            # 0:768) and its HWDGE generation runs while the final split
            # chunk computes; the tail store is only 64KB
            obf = ob[:].rearrange("p m fh f -> p (m fh f)")
            nc.sync.dma_start(out=out_d.ap()[:, : 3 * FH], in_=obf[:, : 3 * FH])
            nc.sync.dma_start(out=out_d.ap()[:, 3 * FH :], in_=obf[:, 3 * FH :])

    # attach the pre-context DMA's completion wait to the first PE
    # instruction (in final scheduled order): PE executes in order, so
    # every subsequent jt0 reader is covered.
    done = False
    for bb in nc.m.functions[0].blocks:
        if done:
            break
        for i in bb.instructions:
            if str(i.engine) == "EngineType.PE" and str(i.opcode) in (
                "Matmult",
                "Ldweights",
            ):
                w = mybir.SyncWait(
                    sync_type="semaphore",
                    id=jt0_sem.num,
                    ant_name="jt0_dma",
                    wait_mode="sem-ge-imm",
                    wait_value=16,
                )
                if i.sync_info is None:
                    i.sync_info = mybir.SyncInfo(on_wait=[w], on_update=[])
                else:
                    i.sync_info.on_wait.append(w)
                done = True
                break
    assert done, "no PE instruction found for jt0 wait"
    nc.compile()
    return nc


def _get_module():
    if "nc" not in _CACHED:
        _CACHED["nc"] = _build_module()
    return _CACHED["nc"]


def _pack_inputs(x, W_v, mlp_w):
    """Host-side shard + pack (bf16) so every DMA source is contiguous."""
    import ml_dtypes

    bf16 = ml_dtypes.bfloat16
    XWQ = 2 * B * (ES // 4)
    AUXW = F + H
    xs = np.asarray(x, dtype=np.float32).reshape(B, J, E).astype(bf16)
    wv = np.asarray(W_v, dtype=np.float32).reshape(H, J, E).astype(bf16)
    mw = np.asarray(mlp_w, dtype=np.float32).astype(bf16)
    ident = np.eye(H, dtype=bf16)
    in_maps = []
    for c in range(NCORES):
        xwa = np.zeros((J, 4 * XWQ + AUXW), dtype=bf16)
        for q in range(4):
            c0 = 2 * B * EOFF[q]
            eq = EQS[q]
            blk = xwa[:, c0 : c0 + 2 * B * eq].reshape(J, 2, B, eq)
            esl = slice(ES * c + EOFF[q], ES * c + EOFF[q] + eq)
            blk[:, 0] = xs[:, :, esl].transpose(1, 0, 2)  # [j, b, e]
            blk[:, 1] = wv[:, :, esl].transpose(1, 0, 2)  # [j, h, e]
        esl = slice(ES * c, ES * (c + 1))
        # mlp_w^T shard straight from the host: rows 0..63 = e', cols = f;
        # the 16x16 transpose identity rides in the trailing columns
        xwa[:ES, 4 * XWQ : 4 * XWQ + F] = mw[:, esl].T
        xwa[:H, 4 * XWQ + F :] = ident
        in_maps.append({"xwa": xwa})
    return in_maps


def run(x, W_v, mlp_w, mlp_b, trace=False, **spmd_kwargs):
    from concourse.bass_utils import run_bass_kernel_spmd

    nc = _get_module()
    in_maps = _pack_inputs(x, W_v, mlp_w)
    res = run_bass_kernel_spmd(
        nc, in_maps, core_ids=list(range(NCORES)), trace=trace, **spmd_kwargs
    )
    partial = np.zeros((B * H, F), dtype=np.float32)
    for r in res.results:
        o = np.asarray(r["out"], dtype=np.float32)
        partial[:128] += o[:, :F]
        partial[128:] += o[:, F:]
    out = partial + np.asarray(mlp_b, dtype=np.float32)[None, :]
    return out.reshape(B, H, F), res


def kernel(x, W_q=None, W_k=None, W_v=None, mlp_w=None, mlp_b=None, **_unused):
    # W_q / W_k are mathematically dead (softmax over the summed axis).
    out, _ = run(x, W_v, mlp_w, mlp_b, trace=False)
    return out
